# revision 1
# baseline (speedup 1.0000x reference)
"""Blockwise 8x8 2D DCT (forward/inverse) on 8 TRN2 NeuronCores.

Reference op: x [B,C,H,W] -> per 8x8 block X: D @ X @ D^T (forward) or
D^T @ X @ D (inverse), with D the 8x8 orthonormal DCT-II matrix.

Strategy (pure data-parallel, batch-sharded across 8 cores):
  Per core the shard is viewed as [rows, W] with rows = (B/8)*C*H.
  For each 128x128 SBUF chunk C the TensorEngine computes
      P1 = C.T @ G        (matmul with C as the stationary operand)
      P2 = P1.T @ G       (matmul with P1 as the stationary operand)
  where G = kron(I_16, Ds.T) is block-diagonal (Ds = D or D.T).  The first
  matmul applies the row (H) transform and transposes the chunk; the second
  applies the column (W) transform and transposes it back.  No explicit
  transposes, 2 matmuls per chunk, all arithmetic in fp32 with fp32 PSUM
  accumulation.

Must be built as bacc.Bacc + nc.compile(): the compile pass legalizes
multi-wait instructions into InstEventSemaphore carriers; raw bass.Bass
programs with >1 sync wait on a Matmult fail walrus codegen.
"""

import numpy as np
from contextlib import ExitStack

P = 128
N_CORES = 8
BLOCK = 8

# best measured configuration (hw-loop slope A/B on silicon)
BEST = dict(wide_dma=2, batch=4)


def _build_nc(
    rows: int,
    width: int,
    repeat: int = 1,
    col_tile: bool = False,
    bufs: int = 4,
    out_ring_scalar: bool = False,
    memcpy_only: bool = False,
    s1_dve: bool = False,
    batch: int = 1,
    hw_loop: int = 0,
    wide_dma: int = 0,
    psum_dma: bool = False,
):
    # wide_dma: number of row-tiles per DMA (0/1 = one tile per DMA)
    # `repeat` re-runs the whole loop inside one NEFF (same output written
    # each time) — used by test.py to measure pure silicon time as a slope
    # between repeat=1 and repeat=R without per-dispatch overhead.
    import concourse.bacc as bacc
    import concourse.mybir as mybir
    import concourse.tile as tile

    nc = bacc.Bacc("TRN2", target_bir_lowering=False, debug=False)
    x = nc.dram_tensor("x", [rows, width], mybir.dt.float32, kind="ExternalInput").ap()
    g = nc.dram_tensor("g", [P, P], mybir.dt.float32, kind="ExternalInput").ap()
    out = nc.dram_tensor(
        "out", [rows, width], mybir.dt.float32, kind="ExternalOutput"
    ).ap()

    n_tiles = rows // P
    n_ch = width // P

    with ExitStack() as ctx:
        tc = ctx.enter_context(tile.TileContext(nc))
        const = ctx.enter_context(tc.tile_pool(name="const", bufs=1))
        xp = ctx.enter_context(tc.tile_pool(name="xp", bufs=bufs))
        op = ctx.enter_context(tc.tile_pool(name="op", bufs=bufs))
        s1p = ctx.enter_context(tc.tile_pool(name="s1p", bufs=8))
        # PSUM is 8 banks of 512 f32; keep p1+p2 pools within 8 banks total.
        p_bufs = 4 if batch <= 4 else 8 // (2 * (batch // 4))
        p1p = ctx.enter_context(tc.tile_pool(name="p1p", bufs=p_bufs, space="PSUM"))
        p2p = ctx.enter_context(tc.tile_pool(name="p2p", bufs=p_bufs, space="PSUM"))

        g_t = const.tile([P, P], mybir.dt.float32)
        nc.sync.dma_start(out=g_t[:], in_=g)

        S = 2 if wide_dma is True else max(int(wide_dma), 1)  # row-tiles/DMA
        if S > 1:
            # [n_tiles/S, P, S, width] view: one DMA moves S row-tiles
            xw = x.rearrange("(a s p) w -> a p s w", p=P, s=S)
            outw = out.rearrange("(a s p) w -> a p s w", p=P, s=S)

        def tile_body(t):
            if S > 1:
                x_t = xp.tile([P, S, width], mybir.dt.float32)
                nc.sync.dma_start(out=x_t[:], in_=xw[t])
                x_views = [x_t[:, s, :] for s in range(S)]
            else:
                x_t = xp.tile([P, width], mybir.dt.float32)
                nc.sync.dma_start(out=x_t[:], in_=x[t * P : (t + 1) * P, :])
                x_views = [x_t[:]]
            if memcpy_only:
                # timing control: same DMA traffic, no compute
                if S > 1:
                    nc.sync.dma_start(out=outw[t], in_=x_t[:])
                else:
                    nc.sync.dma_start(
                        out=out[t * P : (t + 1) * P, :], in_=x_t[:]
                    )
                return
            if not psum_dma:
                if S > 1:
                    o_t = op.tile([P, S, width], mybir.dt.float32)
                    o_views = [o_t[:, s, :] for s in range(S)]
                else:
                    o_t = op.tile([P, width], mybir.dt.float32)
                    o_views = [o_t[:]]

            def mm(dst, src):
                # dst(PSUM) = src(SBUF).T @ g_t
                if not col_tile:
                    nc.tensor.matmul(
                        dst[:], lhsT=src, rhs=g_t[:], start=True, stop=True
                    )
                else:
                    # 4 concurrent M=32 col-group matmuls: 32-column
                    # LDWEIGHTS (27ns vs 107ns) and per-subarray overlap.
                    for ct in range(4):
                        nc.tensor.matmul(
                            dst[32 * ct : 32 * (ct + 1), :],
                            lhsT=src[:, 32 * ct : 32 * (ct + 1)],
                            rhs=g_t[:],
                            tile_position=(0, 32 * ct),
                            start=True,
                            stop=True,
                        )

            for s in range(S):
                xv = x_views[s]
                row0 = (t * S + s) * P
                if batch == 1:
                    assert not psum_dma
                    ov = o_views[s]
                    for j in range(n_ch):
                        p1 = p1p.tile([P, P], mybir.dt.float32)
                        mm(p1, xv[:, j * P : (j + 1) * P])
                        s1 = s1p.tile([P, P], mybir.dt.float32)
                        if s1_dve:
                            nc.vector.tensor_copy(s1[:], p1[:])
                        else:
                            nc.scalar.copy(s1[:], p1[:])
                        p2 = p2p.tile([P, P], mybir.dt.float32)
                        mm(p2, s1[:])
                        nc.vector.tensor_copy(ov[:, j * P : (j + 1) * P], p2[:])
                else:
                    # Pack `batch` chunks' matmul outputs into one PSUM bank
                    # ([128, batch*128] <= one 2KB bank for batch<=4), evict
                    # with a single wide copy (or DMA straight from PSUM).
                    BW = batch * P
                    for jb in range(n_ch // batch):
                        p1 = p1p.tile([P, BW], mybir.dt.float32)
                        for c in range(batch):
                            j = jb * batch + c
                            mm(
                                p1[:, c * P : (c + 1) * P],
                                xv[:, j * P : (j + 1) * P],
                            )
                        s1 = s1p.tile([P, BW], mybir.dt.float32)
                        if s1_dve:
                            nc.vector.tensor_copy(s1[:], p1[:])
                        else:
                            nc.scalar.copy(s1[:], p1[:])
                        p2 = p2p.tile([P, BW], mybir.dt.float32)
                        for c in range(batch):
                            mm(
                                p2[:, c * P : (c + 1) * P],
                                s1[:, c * P : (c + 1) * P],
                            )
                        if psum_dma:
                            nc.sync.dma_start(
                                out=out[
                                    row0 : row0 + P, jb * BW : (jb + 1) * BW
                                ],
                                in_=p2[:],
                            )
                        else:
                            nc.vector.tensor_copy(
                                o_views[s][:, jb * BW : (jb + 1) * BW], p2[:]
                            )
            if not psum_dma:
                out_eng = nc.scalar if out_ring_scalar else nc.sync
                if S > 1:
                    out_eng.dma_start(out=outw[t], in_=o_t[:])
                else:
                    out_eng.dma_start(
                        out=out[t * P : (t + 1) * P, :], in_=o_t[:]
                    )

        n_body = n_tiles // S
        if hw_loop:
            # hardware loop over identical repeats — used for robust timing
            # slopes between two loop counts (floor/overheads cancel).
            with tc.For_i(0, hw_loop, 1):
                for t in range(n_body):
                    tile_body(t)
        else:
            for _ in range(repeat):
                for t in range(n_body):
                    tile_body(t)
    nc.compile()
    return nc


def _make_g(dct_mat: np.ndarray, inverse: int) -> np.ndarray:
    D = np.asarray(dct_mat, dtype=np.float32)
    Ds = D if inverse == 0 else D.T
    return np.kron(
        np.eye(P // Ds.shape[0], dtype=np.float32),
        np.ascontiguousarray(Ds.T, dtype=np.float32),
    )


def _run(x, dct_mat, inverse=0, trace=False):
    from concourse.bass_utils import run_bass_kernel_spmd

    x = np.ascontiguousarray(np.asarray(x, dtype=np.float32))
    inv = int(np.asarray(inverse))
    G = _make_g(dct_mat, inv)

    B, C, H, W = x.shape
    per = B // N_CORES
    rows = per * C * H
    shards = x.reshape(N_CORES, rows, W)

    nc = _build_nc(rows, W, **BEST)
    in_maps = [{"x": shards[i], "g": G} for i in range(N_CORES)]
    res = run_bass_kernel_spmd(
        nc, in_maps, core_ids=list(range(N_CORES)), trace=trace
    )
    y = np.stack([res.results[i]["out"] for i in range(N_CORES)], axis=0)
    return y.reshape(B, C, H, W), res


def kernel(x, dct_mat, inverse=0, **_unused):
    y, _ = _run(x, dct_mat, inverse=inverse, trace=False)
    return y



# revision 5
# speedup vs baseline: 2.2169x; 2.2169x over previous
"""Blockwise 8x8 2D DCT (forward/inverse) on 8 TRN2 NeuronCores.

Reference op: x [B,C,H,W] -> per 8x8 block X: D @ X @ D^T (forward) or
D^T @ X @ D (inverse), with D the 8x8 orthonormal DCT-II matrix.

Strategy (pure data-parallel, batch-sharded across 8 cores):
  Per core the shard is viewed as [rows, W] with rows = (B/8)*C*H.
  For each 128x128 SBUF chunk C the TensorEngine computes
      P1 = C.T @ G        (matmul with C as the stationary operand)
      P2 = P1.T @ G       (matmul with P1 as the stationary operand)
  where G = kron(I_16, Ds.T) is block-diagonal (Ds = D or D.T).  The first
  matmul applies the row (H) transform and transposes the chunk; the second
  applies the column (W) transform and transposes it back.  No explicit
  transposes, 2 matmuls per chunk, all arithmetic in fp32 with fp32 PSUM
  accumulation.

Must be built as bacc.Bacc + nc.compile(): the compile pass legalizes
multi-wait instructions into InstEventSemaphore carriers; raw bass.Bass
programs with >1 sync wait on a Matmult fail walrus codegen.
"""

import numpy as np
import ml_dtypes
from contextlib import ExitStack

P = 128
N_CORES = 8
BLOCK = 8

# best measured configuration (hw-loop slope A/B on silicon)
# bf16 halves HBM traffic (the rel-err budget is 2e-2; bf16 end-to-end is
# ~3e-3) and runs the PE at 1 cycle/row instead of fp32's 4.
BEST = dict(wide_dma=2, batch=4, io_dtype="bf16")


def _build_nc(
    rows: int,
    width: int,
    repeat: int = 1,
    col_tile: bool = False,
    bufs: int = 4,
    out_ring_scalar: bool = False,
    memcpy_only: bool = False,
    s1_dve: bool = False,
    batch: int = 1,
    hw_loop: int = 0,
    wide_dma: int = 0,
    psum_dma: bool = False,
    io_dtype: str = "f32",
):
    # wide_dma: number of row-tiles per DMA (0/1 = one tile per DMA)
    # `repeat` re-runs the whole loop inside one NEFF (same output written
    # each time) — used by test.py to measure pure silicon time as a slope
    # between repeat=1 and repeat=R without per-dispatch overhead.
    import concourse.bacc as bacc
    import concourse.mybir as mybir
    import concourse.tile as tile

    dt = mybir.dt.bfloat16 if io_dtype == "bf16" else mybir.dt.float32

    nc = bacc.Bacc("TRN2", target_bir_lowering=False, debug=False)
    x = nc.dram_tensor("x", [rows, width], dt, kind="ExternalInput").ap()
    g = nc.dram_tensor("g", [P, P], dt, kind="ExternalInput").ap()
    out = nc.dram_tensor(
        "out", [rows, width], dt, kind="ExternalOutput"
    ).ap()

    n_tiles = rows // P
    n_ch = width // P

    with ExitStack() as ctx:
        tc = ctx.enter_context(tile.TileContext(nc))
        const = ctx.enter_context(tc.tile_pool(name="const", bufs=1))
        xp = ctx.enter_context(tc.tile_pool(name="xp", bufs=bufs))
        op = ctx.enter_context(tc.tile_pool(name="op", bufs=bufs))
        s1p = ctx.enter_context(tc.tile_pool(name="s1p", bufs=8))
        # PSUM is 8 banks of 512 f32; keep p1+p2 pools within 8 banks total.
        p_bufs = 4 if batch <= 4 else 8 // (2 * (batch // 4))
        p1p = ctx.enter_context(tc.tile_pool(name="p1p", bufs=p_bufs, space="PSUM"))
        p2p = ctx.enter_context(tc.tile_pool(name="p2p", bufs=p_bufs, space="PSUM"))

        g_t = const.tile([P, P], dt)
        nc.sync.dma_start(out=g_t[:], in_=g)

        S = 2 if wide_dma is True else max(int(wide_dma), 1)  # row-tiles/DMA
        if S > 1:
            # [n_tiles/S, P, S, width] view: one DMA moves S row-tiles
            xw = x.rearrange("(a s p) w -> a p s w", p=P, s=S)
            outw = out.rearrange("(a s p) w -> a p s w", p=P, s=S)

        def tile_body(t):
            if S > 1:
                x_t = xp.tile([P, S, width], dt)
                nc.sync.dma_start(out=x_t[:], in_=xw[t])
                x_views = [x_t[:, s, :] for s in range(S)]
            else:
                x_t = xp.tile([P, width], dt)
                nc.sync.dma_start(out=x_t[:], in_=x[t * P : (t + 1) * P, :])
                x_views = [x_t[:]]
            if memcpy_only:
                # timing control: same DMA traffic, no compute
                if S > 1:
                    nc.sync.dma_start(out=outw[t], in_=x_t[:])
                else:
                    nc.sync.dma_start(
                        out=out[t * P : (t + 1) * P, :], in_=x_t[:]
                    )
                return
            if not psum_dma:
                if S > 1:
                    o_t = op.tile([P, S, width], dt)
                    o_views = [o_t[:, s, :] for s in range(S)]
                else:
                    o_t = op.tile([P, width], dt)
                    o_views = [o_t[:]]

            def mm(dst, src):
                # dst(PSUM) = src(SBUF).T @ g_t
                if not col_tile:
                    nc.tensor.matmul(
                        dst[:], lhsT=src, rhs=g_t[:], start=True, stop=True
                    )
                else:
                    # 4 concurrent M=32 col-group matmuls: 32-column
                    # LDWEIGHTS (27ns vs 107ns) and per-subarray overlap.
                    for ct in range(4):
                        nc.tensor.matmul(
                            dst[32 * ct : 32 * (ct + 1), :],
                            lhsT=src[:, 32 * ct : 32 * (ct + 1)],
                            rhs=g_t[:],
                            tile_position=(0, 32 * ct),
                            start=True,
                            stop=True,
                        )

            for s in range(S):
                xv = x_views[s]
                row0 = (t * S + s) * P
                if batch == 1:
                    assert not psum_dma
                    ov = o_views[s]
                    for j in range(n_ch):
                        p1 = p1p.tile([P, P], mybir.dt.float32)
                        mm(p1, xv[:, j * P : (j + 1) * P])
                        s1 = s1p.tile([P, P], dt)
                        if s1_dve:
                            nc.vector.tensor_copy(s1[:], p1[:])
                        else:
                            nc.scalar.copy(s1[:], p1[:])
                        p2 = p2p.tile([P, P], mybir.dt.float32)
                        mm(p2, s1[:])
                        nc.vector.tensor_copy(ov[:, j * P : (j + 1) * P], p2[:])
                else:
                    # Pack `batch` chunks' matmul outputs into one PSUM bank
                    # ([128, batch*128] <= one 2KB bank for batch<=4), evict
                    # with a single wide copy (or DMA straight from PSUM).
                    BW = batch * P
                    for jb in range(n_ch // batch):
                        p1 = p1p.tile([P, BW], mybir.dt.float32)
                        for c in range(batch):
                            j = jb * batch + c
                            mm(
                                p1[:, c * P : (c + 1) * P],
                                xv[:, j * P : (j + 1) * P],
                            )
                        s1 = s1p.tile([P, BW], dt)
                        if s1_dve:
                            nc.vector.tensor_copy(s1[:], p1[:])
                        else:
                            nc.scalar.copy(s1[:], p1[:])
                        p2 = p2p.tile([P, BW], mybir.dt.float32)
                        for c in range(batch):
                            mm(
                                p2[:, c * P : (c + 1) * P],
                                s1[:, c * P : (c + 1) * P],
                            )
                        if psum_dma:
                            nc.sync.dma_start(
                                out=out[
                                    row0 : row0 + P, jb * BW : (jb + 1) * BW
                                ],
                                in_=p2[:],
                            )
                        else:
                            nc.vector.tensor_copy(
                                o_views[s][:, jb * BW : (jb + 1) * BW], p2[:]
                            )
            if not psum_dma:
                out_eng = nc.scalar if out_ring_scalar else nc.sync
                if S > 1:
                    out_eng.dma_start(out=outw[t], in_=o_t[:])
                else:
                    out_eng.dma_start(
                        out=out[t * P : (t + 1) * P, :], in_=o_t[:]
                    )

        n_body = n_tiles // S
        if hw_loop:
            # hardware loop over identical repeats — used for robust timing
            # slopes between two loop counts (floor/overheads cancel).
            with tc.For_i(0, hw_loop, 1):
                for t in range(n_body):
                    tile_body(t)
        else:
            for _ in range(repeat):
                for t in range(n_body):
                    tile_body(t)
    nc.compile()
    return nc


def _make_g(dct_mat: np.ndarray, inverse: int) -> np.ndarray:
    D = np.asarray(dct_mat, dtype=np.float32)
    Ds = D if inverse == 0 else D.T
    return np.kron(
        np.eye(P // Ds.shape[0], dtype=np.float32),
        np.ascontiguousarray(Ds.T, dtype=np.float32),
    )


def _run(x, dct_mat, inverse=0, trace=False):
    from concourse.bass_utils import run_bass_kernel_spmd

    x = np.ascontiguousarray(np.asarray(x, dtype=np.float32))
    inv = int(np.asarray(inverse))
    G = _make_g(dct_mat, inv)
    if BEST.get("io_dtype") == "bf16":
        x = x.astype(ml_dtypes.bfloat16)
        G = G.astype(ml_dtypes.bfloat16)

    B, C, H, W = x.shape
    per = B // N_CORES
    rows = per * C * H
    shards = x.reshape(N_CORES, rows, W)

    nc = _build_nc(rows, W, **BEST)
    in_maps = [{"x": shards[i], "g": G} for i in range(N_CORES)]
    res = run_bass_kernel_spmd(
        nc, in_maps, core_ids=list(range(N_CORES)), trace=trace
    )
    y = np.stack([res.results[i]["out"] for i in range(N_CORES)], axis=0)
    return y.reshape(B, C, H, W).astype(np.float32), res


def kernel(x, dct_mat, inverse=0, **_unused):
    y, _ = _run(x, dct_mat, inverse=inverse, trace=False)
    return y

